# revision 16
# baseline (speedup 1.0000x reference)
"""Trainium2 Bass kernel for nn_CA_Model (neural cellular automaton).

Strategy: pure data-parallel over batch (8 images -> 8 cores). Per core, the
whole [256,256,16] image lives in SBUF in an interleaved layout
  X[p, f]: p = (row%8)*16 + ch, f = (row//8)*258 + 1 + w
with zero padding columns (w=-1,256) and zero pad rows (row 0, 257; image rows
are 1..256). The 3x3 depthwise perceive + first MLP layer fuse into three
PSUM-accumulated matmuls per output row (one per horizontal tap dj): the
vertical taps come from the lhsT's placement of per-tap weight blocks
A[di,dj] on the partition rows of the enclosing 8-row block.

Precision split: the master state XN ping-pongs in f32 (d accumulates in f32,
life mask applied in place); a bf16 shadow copy Xb feeds the matmuls so the
tensor engine gets FWL fast weight loads (fp32 weights would pay ~128 cycles
of LDWEIGHTS per matmul). Layer-2 and life-broadcast weights are bf16 too.

Half-1 life matmuls of step t are emitted after the first few sweep
iterations of step t+1 so the in-order tensor queue is not stalled behind
the post-pool DVE/DMA chain at the step boundary.
"""
import sys
for _p in ("/opt/trn_rl_repo", "/root/.axon_site/_ro/trn_rl_repo"):
    if _p not in sys.path:
        sys.path.append(_p)

import numpy as np

C = 16
HID = 128
H = W = 256
NB = 33            # row blocks in X layout (33*8 = 264 row slots, rows 0..257 used)
FW = 258           # padded row width in free dim
FSZ = NB * FW      # 8514 free elements per partition
NROW = 256         # image rows (stored at row index 1..256)


def _sobel():
    dx = np.outer([1, 2, 1], [-1, 0, 1]) / 8.0
    f1 = dx.T.astype(np.float32)   # angle=0: F1 = dx.T
    f2 = dx.astype(np.float32)     # F2 = dx
    return f1, f2


def build_weights(W0, b0, W1):
    """Host-side preprocessing of the MLP weights into lhsT tensors."""
    F1, F2 = _sobel()
    W0x, W0y1, W0y2 = W0[:, 0:16], W0[:, 16:32], W0[:, 32:48]
    # A[di][dj]: [HID, C] applied to x[row-1+di, w-1+dj]
    A = [[(np.float32(di == 1 and dj == 1) * W0x
           + F1[di, dj] * W0y1 + F2[di, dj] * W0y2).astype(np.float32)
          for dj in range(3)] for di in range(3)]

    # layer-1 lhsT variants. Key: (s, dj, part) where part is 0 for the lhsT
    # covering the block holding row rho-1 (and possibly rho, rho+1), 1 for
    # the spill into block t+1 (s in {6,7}).
    l1 = {}
    for dj in range(3):
        for s in range(6):
            L = np.zeros((128, 128), np.float32)
            for di in range(3):
                # lhsT[16*(s+di)+c, m] = A[di][dj][m, c]
                L[16 * (s + di):16 * (s + di) + 16, :] = A[di][dj].T
            l1[(s, dj, 0)] = L
        # s in {6, 7}: single matmul against the X2 (4-row-shifted blocking)
        # copy. Row r sits at X2 partition group (r+4)%8: s=6 -> rows at
        # groups 2,3,4; s=7 -> groups 3,4,5.
        for s, g0 in ((6, 2), (7, 3)):
            L = np.zeros((128, 128), np.float32)
            for di in range(3):
                L[16 * (g0 + di):16 * (g0 + di) + 16, :] = A[di][dj].T
            l1[(s, dj, 0)] = L

    # layer-2 lhsT: two M=32 variants shared by all col-groups. A row with
    # group g lands in col-group j=g//2 at partition stripe 32j+16*(g%2).
    # W1E covers even g (stripe low half), W1O odd g (high half); the unused
    # half is zero (accumulates 0 into the partner stripe).
    w1p = []
    for par in range(2):
        Wp = np.zeros((128, 32), np.float32)
        Wp[:, 16 * par:16 * par + 16] = W1.T
        w1p.append(Wp)

    # life-broadcast lhsT variants: for block tb, rows rho=8tb+g (real rows
    # only), life value lives at LifeQ[q = rho-1, half = q//128].
    life_plan = []   # per tb: list of (half, lhsT_index); lhsT K=128 base 0
    r_mats = []
    for tb in range(NB):
        plan = []
        buckets = {}
        for g in range(8):
            rho = 8 * tb + g
            if rho < 1 or rho > 256:
                continue
            q = rho - 1
            half, qh = q // 128, q % 128
            buckets.setdefault(half, []).append((g, qh))
        for half, rows in sorted(buckets.items()):
            # lhsT indexed by the pool tile's q2 layout: row (q%8)*16 + (q//8)%16
            Rm = np.zeros((128, 128), np.float32)
            for g, qh in rows:
                q2 = (qh % 8) * 16 + qh // 8
                Rm[q2, 16 * g:16 * g + 16] = 1.0
            plan.append((half, len(r_mats)))
            r_mats.append(Rm)
        life_plan.append(plan)

    import ml_dtypes
    bf16 = ml_dtypes.bfloat16
    l1_keys = sorted(l1.keys())
    l1_idx = {k: i for i, k in enumerate(l1_keys)}
    l1_stack = np.stack([l1[k] for k in l1_keys]).astype(bf16)  # [NL1,128,128]
    w1_stack = np.stack(w1p).astype(bf16)                  # [2, 128, 32] bf16
    r_stack = np.stack(r_mats).astype(bf16)                # [NR, 128, 128] bf16
    return dict(l1_stack=l1_stack, l1_idx=l1_idx, w1_stack=w1_stack,
                r_stack=r_stack, life_plan=life_plan,
                b0=b0.reshape(128, 1).astype(np.float32))


def marshal_x2(xc):
    """Shifted-blocking copy: row r at partition ((r+4)%8)*16+c, block (r+4)//8."""
    x2 = np.zeros_like(xc)
    x2[64:128, :] = xc[0:64, :]
    x2[0:64, FW:] = xc[64:128, :-FW]
    return x2


def marshal_x(img):
    """[256,256,16] image -> X [128, FSZ] interleaved layout (f32)."""
    xp = np.zeros((NB * 8, FW, C), np.float32)
    xp[1:257, 1:257, :] = img
    # X[g*16+c, t*258+w] = xp[8t+g, w, c]
    xc = xp.reshape(NB, 8, FW, C).transpose(1, 3, 0, 2).reshape(128, FSZ)
    return np.ascontiguousarray(xc)


def unmarshal_x(xc):
    """X [128, FSZ] -> [256,256,16] image."""
    xp = xc.reshape(8, C, NB, FW).transpose(2, 0, 3, 1)   # [NB, 8, FW, C]
    xp = xp.reshape(NB * 8, FW, C)
    return np.ascontiguousarray(xp[1:257, 1:257, :])


def build_program(steps, l1_idx, life_plan, n_l1, n_r,
                  relu_act_ratio=4, defer_tp=3, defer_trios=3):
    """Returns a compiled Bacc ready for run_bass_kernel_spmd.

    relu_act_ratio: of every 4 relu tiles, this many go to ScalarE (rest DVE).
    defer_tp: sweep iteration of step t+1 after which step t's half-1 life
    chain is emitted (so the tensor queue isn't stalled on the post pools).
    """
    import concourse.bass as bass
    import concourse.bacc as bacc
    import concourse.tile as tile
    from concourse import mybir
    F32 = mybir.dt.float32
    BF16 = mybir.dt.bfloat16
    AF = mybir.ActivationFunctionType
    ALU = mybir.AluOpType
    nc = bacc.Bacc(None, target_bir_lowering=False, debug=False, num_devices=8,
                   num_swdge_queues=4)

    xb_ext = nc.declare_dram_parameter("xbc", [128, FSZ], BF16, isOutput=False)
    x2_ext = nc.declare_dram_parameter("x2c", [128, FSZ], BF16, isOutput=False)
    l1_ext = nc.declare_dram_parameter("l1w", [n_l1, 128, 128], BF16, isOutput=False)
    w1_ext = nc.declare_dram_parameter("w1w", [2, 128, 32], BF16, isOutput=False)
    r_ext = nc.declare_dram_parameter("rw", [n_r, 128, 128], BF16, isOutput=False)
    b0_ext = nc.declare_dram_parameter("b0w", [128, 1], F32, isOutput=False)
    out_ext = nc.declare_dram_parameter("out", [128, FSZ], F32, isOutput=True)

    with tile.TileContext(nc) as tc:
        with tc.tile_pool(name="hpool", bufs=12) as hpool, \
             tc.tile_pool(name="ph_pool", bufs=3, space="PSUM") as ph_pool, \
             tc.tile_pool(name="pd_pool", bufs=3, space="PSUM") as pd_pool, \
             tc.tile_pool(name="pl_pool", bufs=2, space="PSUM") as pl_pool:

            # --- persistent state (static SBUF allocations, never released) ---
            XN_A = nc.alloc_sbuf_tensor("XN_A", [128, FSZ], F32).ap()
            XN_B = nc.alloc_sbuf_tensor("XN_B", [128, FSZ], F32).ap()
            Xb_A = nc.alloc_sbuf_tensor("Xb_A", [128, FSZ], BF16).ap()
            Xb_B = nc.alloc_sbuf_tensor("Xb_B", [128, FSZ], BF16).ap()
            X2b = nc.alloc_sbuf_tensor("X2b", [128, FSZ], BF16).ap()

            LW1 = nc.alloc_sbuf_tensor("LW1", [128, n_l1 * 128], BF16).ap()
            LW2 = nc.alloc_sbuf_tensor("LW2", [128, 2 * 32], BF16).ap()
            LWR = nc.alloc_sbuf_tensor("LWR", [128, n_r * 128], BF16).ap()
            l1t = [LW1[:, 128 * i:128 * i + 128] for i in range(n_l1)]
            w1t = [LW2[:, 32 * par:32 * par + 32] for par in range(2)]
            rt = [LWR[:, 128 * i:128 * i + 128] for i in range(n_r)]
            b0t = nc.alloc_sbuf_tensor("b0t", [128, 1], F32).ap()

            A_pre = nc.alloc_sbuf_tensor("A_pre", [128, 516], F32).ap()
            A_post = nc.alloc_sbuf_tensor("A_post", [128, 516], F32).ap()
            HM = nc.alloc_sbuf_tensor("HM", [128, 512], F32).ap()
            HMu = nc.alloc_sbuf_tensor("HMu", [128, 512], F32).ap()
            HMd = nc.alloc_sbuf_tensor("HMd", [128, 512], F32).ap()
            HMp = nc.alloc_sbuf_tensor("HMp", [128, 512], F32).ap()
            HMpu = nc.alloc_sbuf_tensor("HMpu", [128, 512], F32).ap()
            HMpd = nc.alloc_sbuf_tensor("HMpd", [128, 512], F32).ap()
            # seam scratch: partition 0 holds alpha/hm of rows 128,129
            # cols: [0:258] a128 | [258:516] a129 | [516:774] hm128 | [774:1032] hm129
            SEAM = nc.alloc_sbuf_tensor("SEAM", [128, 1032], F32).ap()
            VMpre = nc.alloc_sbuf_tensor("VMpre", [128, 512], F32).ap()
            VMpost = nc.alloc_sbuf_tensor("VMpost", [128, 512], F32).ap()
            LifeQ = nc.alloc_sbuf_tensor("LifeQ", [128, 512], BF16).ap()
            LifeF = nc.alloc_sbuf_tensor("LifeF", [128, 512], F32).ap()
            Zrow = nc.alloc_sbuf_tensor("Zrow", [128, 516], F32).ap()

            # --- loads / init ---
            # Ordered so the first sweep iteration unblocks as early as
            # possible: image chunk 0 + layer-1 weights first; the big
            # life-broadcast weights (LWR, only needed mid-sweep) last.
            # Image tensors load in block-chunks so the sweep starts once
            # chunk 0 lands instead of waiting for the full 5MB.
            first = True
            for t0, nt in ((0, 8), (8, 8), (16, 8), (24, 9)):
                lo, n = t0 * FW, nt * FW
                nc.gpsimd.dma_start(out=Xb_A[:, lo:lo + n],
                                    in_=xb_ext[:, lo:lo + n])
                nc.gpsimd.dma_start(out=X2b[:, lo:lo + n],
                                    in_=x2_ext[:, lo:lo + n])
                # f32 master derived on-device from the bf16 input
                nc.vector.tensor_copy(XN_A[:, lo:lo + n], Xb_A[:, lo:lo + n])
                if first:
                    first = False
                    nc.gpsimd.dma_start(out=LW1[:], in_=bass.AP(
                        tensor=l1_ext, offset=0,
                        ap=[[128, 128], [128 * 128, n_l1], [1, 128]]))
                    nc.gpsimd.dma_start(out=b0t[:], in_=b0_ext[:])
                    nc.gpsimd.dma_start(out=LW2[:], in_=bass.AP(
                        tensor=w1_ext, offset=0,
                        ap=[[32, 128], [128 * 32, 2], [1, 32]]))
            nc.gpsimd.dma_start(out=LWR[:], in_=bass.AP(
                tensor=r_ext, offset=0,
                ap=[[128, 128], [128 * 128, n_r], [1, 128]]))
            # only the pad columns of the Xb shadow need zeroing (interior
            # rows are fully rewritten each step; host tensors arrive padded)
            nc.vector.memset(bass.AP(tensor=Xb_B.tensor, offset=0,
                                     ap=[[FSZ, 128], [FW, NB], [257, 2]]), 0.0)
            nc.vector.memset(Zrow[:], 0.0)
            nc.vector.memset(SEAM[0:32, :], 0.0)
            nc.vector.memset(A_post[:], 0.0)
            nc.vector.memset(A_pre[:], 0.0)

            relu_ctr = [0]

            def relu_tile(dst, src):
                use_act = (relu_ctr[0] % 4) < relu_act_ratio
                relu_ctr[0] += 1
                if use_act:
                    nc.scalar.activation(dst, src, AF.Relu, bias=b0t[:], scale=1.0)
                else:
                    nc.vector.tensor_scalar(dst, src, b0t[:], 0.0,
                                            op0=ALU.add, op1=ALU.max)

            def extract_alpha(dst_A, src_X, halves=(0, 1)):
                # q2-layout: dst_A[q2 = gp*16 + j, half*258 + 1 + w] holds
                # alpha of row rho = 128*half + 8j + gp + 1.
                for half in halves:
                    for gp in range(8):
                        g = (gp + 1) % 8
                        t0 = 16 * half + (1 if gp == 7 else 0)
                        dst = bass.AP(
                            tensor=dst_A.tensor,
                            offset=16 * gp * 516 + 258 * half + 1,
                            ap=[[516, 16], [1, 256]])
                        src = bass.AP(
                            tensor=src_X.tensor,
                            offset=(16 * g + 3) * FSZ + t0 * FW + 1,
                            ap=[[FSZ, 1], [FW, 16], [1, 256]])
                        nc.gpsimd.dma_start(out=dst, in_=src)

            def pool_half(dst_VM, src_A, half, hm, hmu, hmd):
                """maxpool one half; seam values come from the SEAM tile."""
                lo, hi = 258 * half, 258 * half + 258
                qlo, qhi = 256 * half, 256 * half + 256
                av = src_A[:, lo:hi]
                nc.vector.tensor_tensor(hm[:, qlo:qhi], av[:, 0:256],
                                        av[:, 2:258], op=ALU.max)
                nc.vector.tensor_tensor(hm[:, qlo:qhi], hm[:, qlo:qhi],
                                        av[:, 1:257], op=ALU.max)
                nc.gpsimd.dma_start(out=hmu[0:112, qlo:qhi], in_=hm[16:128, qlo:qhi])
                nc.gpsimd.dma_start(out=hmu[112:127, qlo:qhi], in_=hm[1:16, qlo:qhi])
                nc.gpsimd.dma_start(out=hmd[16:128, qlo:qhi], in_=hm[0:112, qlo:qhi])
                nc.gpsimd.dma_start(out=hmd[1:16, qlo:qhi], in_=hm[112:127, qlo:qhi])
                if half == 0:
                    # HMu[127, h0] = hm(row 129); HMd[0, h0] = 0
                    nc.gpsimd.dma_start(out=hmu[127:128, 0:256],
                                        in_=SEAM[0:1, 775:1031])
                    nc.gpsimd.dma_start(out=hmd[0:1, 0:256], in_=Zrow[0:1, 0:256])
                else:
                    # HMd[0, h1] = hm(row 128); HMu[127, h1] = 0
                    nc.gpsimd.dma_start(out=hmd[0:1, 256:512],
                                        in_=SEAM[0:1, 517:773])
                    nc.gpsimd.dma_start(out=hmu[127:128, 256:512],
                                        in_=Zrow[0:1, 0:256])
                nc.vector.tensor_tensor(dst_VM[:, qlo:qhi], hm[:, qlo:qhi],
                                        hmu[:, qlo:qhi], op=ALU.max)
                nc.vector.tensor_tensor(dst_VM[:, qlo:qhi], dst_VM[:, qlo:qhi],
                                        hmd[:, qlo:qhi], op=ALU.max)

            def seam_fill_from_A(src_A):
                # rows 128 (q2=127, h0) / 129 (q2=0, h1) from the A tile
                nc.gpsimd.dma_start(out=SEAM[0:1, 1:257],
                                    in_=src_A[127:128, 1:257])
                nc.gpsimd.dma_start(out=SEAM[0:1, 259:515],
                                    in_=src_A[0:1, 259:515])
                sv = SEAM[0:1, :].rearrange("p (a w) -> p a w", a=4)
                nc.vector.tensor_tensor(sv[:, 2:4, 1:257], sv[:, 0:2, 0:256],
                                        sv[:, 0:2, 2:258], op=ALU.max)
                nc.vector.tensor_tensor(sv[:, 2:4, 1:257], sv[:, 2:4, 1:257],
                                        sv[:, 0:2, 1:257], op=ALU.max)

            def seam_fill(src_X):
                # alpha rows 128 (g 0, t 16) and 129 (g 1, t 16) -> SEAM
                # partition 0; then their horizontal maxes.
                nc.gpsimd.dma_start(
                    out=SEAM[0:1, 1:257],
                    in_=bass.AP(tensor=src_X.tensor,
                                offset=3 * FSZ + 16 * FW + 1,
                                ap=[[FSZ, 1], [1, 256]]))
                nc.gpsimd.dma_start(
                    out=SEAM[0:1, 259:515],
                    in_=bass.AP(tensor=src_X.tensor,
                                offset=(16 + 3) * FSZ + 16 * FW + 1,
                                ap=[[FSZ, 1], [1, 256]]))
                sv = SEAM[0:1, :].rearrange("p (a w) -> p a w", a=4)
                nc.vector.tensor_tensor(sv[:, 2:4, 1:257], sv[:, 0:2, 0:256],
                                        sv[:, 0:2, 2:258], op=ALU.max)
                nc.vector.tensor_tensor(sv[:, 2:4, 1:257], sv[:, 2:4, 1:257],
                                        sv[:, 0:2, 1:257], op=ALU.max)

            def xwin2(xt, t, lo, n):
                # two consecutive blocks, same in-block window -> free (2, n)
                return xt.rearrange("p (t f) -> p t f", t=NB)[:, t:t + 2, lo:lo + n]

            def make_life_block(step, XNc, Xbc, last):
                def life_block(tb):
                    lo = tb * FW + 1
                    plan = life_plan[tb]
                    pl = pl_pool.tile([128, 256], F32,
                                      name=f"pl_{step}_{tb}", tag="pl")
                    for i, (half, ridx) in enumerate(plan):
                        nc.tensor.matmul(
                            pl[:], rt[ridx],
                            LifeQ[:, half * 256:half * 256 + 256],
                            start=(i == 0), stop=(i == len(plan) - 1))
                    # life mask applied in place on the f32 master
                    nc.vector.tensor_tensor(XNc[:, lo:lo + 256],
                                            XNc[:, lo:lo + 256], pl[:],
                                            op=ALU.mult)
                    if last:
                        nc.gpsimd.dma_start(
                            out=out_ext[:, lo:lo + 256],
                            in_=XNc[:, lo:lo + 256])
                    else:
                        # bf16 shadow for the next step's matmuls
                        nc.vector.tensor_copy(Xbc[:, lo:lo + 256],
                                              XNc[:, lo:lo + 256])
                        nc.gpsimd.dma_start(
                            out=X2b[64:128, lo:lo + 256],
                            in_=Xbc[0:64, lo:lo + 256])
                        if tb < NB - 1:
                            nc.gpsimd.dma_start(
                                out=X2b[0:64, lo + FW:lo + FW + 256],
                                in_=Xbc[64:128, lo:lo + 256])
                return life_block

            pending_h1 = [None]   # deferred half-1 life chain of prev step

            for step in range(steps):
                last = step + 1 == steps
                if step % 2 == 0:
                    XNprev, XNcur = XN_A, XN_B
                    Xbprev, Xbcur = Xb_A, Xb_B
                else:
                    XNprev, XNcur = XN_B, XN_A
                    Xbprev, Xbcur = Xb_B, Xb_A
                life_block = make_life_block(step, XNcur, Xbcur, last)

                # --- pre pool (A_pre was refreshed at the end of the
                # previous step, off the critical path) ---
                if step == 0:
                    extract_alpha(A_pre, XNprev)
                seam_fill_from_A(A_pre)
                pool_half(VMpre, A_pre, 0, HM, HMu, HMd)
                pool_half(VMpre, A_pre, 1, HM, HMu, HMd)

                # --- main sweep: layer1 + relu + layer2 ---
                d_tiles = {}
                d_count = {}          # per-block total arrivals
                s_count = {}          # per (tb, colgroup j) stripe arrivals
                d_expect = {tb: 8 for tb in range(NB)}
                d_expect[0] = 7
                d_expect[32] = 1
                s_expect = {}
                for tb in range(NB):
                    for j in range(4):
                        s_expect[(tb, j)] = 2
                s_expect[(0, 0)] = 1     # row 0 is padding; only g=1 writes
                s_expect[(32, 0)] = 1    # block 32 holds only row 256 (g=0)

                def post_half(half):
                    extract_alpha(A_post, XNcur, halves=(half,))
                    pool_half(VMpost, A_post, half, HMp, HMpu, HMpd)
                    qlo = 256 * half
                    qs = slice(qlo, qlo + 256)
                    nc.vector.tensor_tensor(LifeF[:, qs], VMpre[:, qs],
                                            VMpost[:, qs], op=ALU.min)
                    nc.vector.tensor_scalar(LifeF[:, qs], LifeF[:, qs],
                                            0.1, None, op0=ALU.is_gt)
                    nc.vector.tensor_copy(LifeQ[:, qs], LifeF[:, qs])
                    if not last:
                        flo = 258 * half + 1
                        nc.vector.tensor_tensor(A_pre[:, flo:flo + 256],
                                                A_post[:, flo:flo + 256],
                                                LifeF[:, qs], op=ALU.mult)
                    if half == 0:
                        for tb in range(0, 16):
                            life_block(tb)

                def l2mm(rho, ht, hslice):
                    # one [K=128, M=32] matmul into col-group stripe 32j of
                    # the block's d tile. The lhsT (W1E/W1O by row parity)
                    # zero-fills the partner 16-row half, so the stripe's
                    # even/odd pair accumulates into disjoint 16-partition
                    # sub-stripes with 2 matmuls per 32-stripe.
                    tb, g = rho // 8, rho % 8
                    j, par = g // 2, g % 2
                    if tb not in d_tiles:
                        d_tiles[tb] = pd_pool.tile([128, 256], F32,
                                                   name=f"pd_s{step}_{tb}",
                                                   tag="pd")
                        d_count[tb] = 0
                    first = s_count.get((tb, j), 0) == 0
                    s_count[(tb, j)] = s_count.get((tb, j), 0) + 1
                    laststripe = s_count[(tb, j)] == s_expect[(tb, j)]
                    nc.tensor.matmul(d_tiles[tb][32 * j:32 * j + 32, :],
                                     w1t[par][:], ht[:, hslice],
                                     start=first, stop=laststripe,
                                     tile_position=(0, 32 * j))
                    d_count[tb] += 1
                    if d_count[tb] == d_expect[tb]:
                        lo = tb * FW + 1
                        nc.vector.tensor_tensor(
                            XNcur[:, lo:lo + 256], d_tiles[tb][:],
                            XNprev[:, lo:lo + 256], op=ALU.add)
                        if tb == 16:
                            seam_fill(XNcur)
                            post_half(0)

                # Layer-2 runs one whole tp behind layer-1 so (a) the tensor
                # queue never waits on relu latency and (b) the 16 matmuls of
                # a tp can be emitted as 4 quads whose members hit 4 distinct
                # 32-column PE groups -> they execute concurrently.
                pending_l2 = []

                def flush_tp_l2():
                    trios = pending_l2[:8]
                    del pending_l2[:8]
                    for seq, hs in (((0, 2, 4, 6), slice(0, 256)),
                                    ((0, 2, 4, 6), slice(256, 512)),
                                    ((1, 3, 5, 7), slice(0, 256)),
                                    ((1, 3, 5, 7), slice(256, 512))):
                        for s in seq:
                            rho, ht = trios[s]
                            l2mm(rho + (8 if hs.start else 0), ht, hs)

                for tp in range(16):
                    if tp == defer_tp and pending_h1[0] is not None:
                        # previous step's half-1 life chain: its LifeQ deps
                        # are long satisfied by now, so these matmuls flow
                        # without stalling the tensor queue.
                        pending_h1[0]()
                        pending_h1[0] = None
                    # trios are emitted in s-pairs with their dj matmuls
                    # interleaved: consecutive matmuls then target different
                    # PSUM banks, which lets the PE pipeline the fill of one
                    # against the drain of the other (same-bank accumulation
                    # groups serialize at ~N cycles each).
                    for s0 in range(0, 8, 2):
                        phs = {}
                        for dj in range(3):
                            for s in (s0, s0 + 1):
                                if s not in phs:
                                    phs[s] = ph_pool.tile(
                                        [128, 2, 256], F32,
                                        name=f"ph_{step}_{tp}_{s}", tag="ph")
                                src = Xbprev if s <= 5 else X2b
                                tblk = 2 * tp if s <= 5 else 2 * tp + 1
                                nc.tensor.matmul(
                                    phs[s][:], l1t[l1_idx[(s, dj, 0)]][:],
                                    xwin2(src, tblk, dj, 256),
                                    start=(dj == 0), stop=(dj == 2))
                        for s in (s0, s0 + 1):
                            rho1 = 16 * tp + s + 1
                            ht = hpool.tile([128, 512], BF16, tag="ht")
                            relu_tile(ht[:], phs[s].rearrange("p a b -> p (a b)"))
                            pending_l2.append((rho1, ht))
                            if len(pending_l2) == 8 + defer_trios:
                                flush_tp_l2()
                while pending_l2:
                    flush_tp_l2()

                # h0's post chain was emitted mid-sweep (block 16); finish h1
                post_half(1)

                def deferred(lb=life_block):
                    # block 16 first: the next step's sweep needs its Xb/X2b
                    # updates earliest (tp=7..8 read blocks 16-17).
                    for tb in range(16, NB):
                        lb(tb)
                if last:
                    deferred()
                else:
                    pending_h1[0] = deferred

    nc.compile()
    return nc


_PROGRAM_CACHE = {}


def kernel(x, W0, b0, W1, steps, _trace=False):
    import concourse.bass_utils as bass_utils
    import ml_dtypes
    bf16 = ml_dtypes.bfloat16
    steps = int(steps)
    x = np.asarray(x, dtype=np.float32)
    W0 = np.asarray(W0, dtype=np.float32)
    b0 = np.asarray(b0, dtype=np.float32)
    W1 = np.asarray(W1, dtype=np.float32)
    B = x.shape[0]
    assert x.shape == (8, H, W, C), x.shape

    wts = build_weights(W0, b0, W1)
    key = steps
    if key not in _PROGRAM_CACHE:
        _PROGRAM_CACHE[key] = build_program(steps, wts["l1_idx"],
                                            wts["life_plan"],
                                            wts["l1_stack"].shape[0],
                                            wts["r_stack"].shape[0])
    nc = _PROGRAM_CACHE[key]

    in_maps = []
    for b in range(B):
        xcb = marshal_x(x[b])
        xbb = xcb.astype(bf16)
        in_maps.append({
            "xbc": xbb,
            "x2c": marshal_x2(xbb),
            "l1w": wts["l1_stack"],
            "w1w": wts["w1_stack"],
            "rw": wts["r_stack"],
            "b0w": wts["b0"],
        })
    res = bass_utils.run_bass_kernel_spmd(nc, in_maps, list(range(8)),
                                          trace=_trace)
    kernel.last_result = res
    out = np.stack([unmarshal_x(res.results[b]["out"]) for b in range(B)])
    return out.astype(np.float32)



# revision 19
# speedup vs baseline: 1.0127x; 1.0127x over previous
"""Trainium2 Bass kernel for nn_CA_Model (neural cellular automaton).

Strategy: pure data-parallel over batch (8 images -> 8 cores). Per core, the
whole [256,256,16] image lives in SBUF in an interleaved layout
  X[p, f]: p = (row%8)*16 + ch, f = (row//8)*258 + 1 + w
with zero padding columns (w=-1,256) and zero pad rows (row 0, 257; image rows
are 1..256). The 3x3 depthwise perceive + first MLP layer fuse into three
PSUM-accumulated matmuls per output row (one per horizontal tap dj): the
vertical taps come from the lhsT's placement of per-tap weight blocks
A[di,dj] on the partition rows of the enclosing 8-row block.

Precision split: the master state XN ping-pongs in f32 (d accumulates in f32,
life mask applied in place); a bf16 shadow copy Xb feeds the matmuls so the
tensor engine gets FWL fast weight loads (fp32 weights would pay ~128 cycles
of LDWEIGHTS per matmul). Layer-2 and life-broadcast weights are bf16 too.

Half-1 life matmuls of step t are emitted after the first few sweep
iterations of step t+1 so the in-order tensor queue is not stalled behind
the post-pool DVE/DMA chain at the step boundary.
"""
import sys
for _p in ("/opt/trn_rl_repo", "/root/.axon_site/_ro/trn_rl_repo"):
    if _p not in sys.path:
        sys.path.append(_p)

import numpy as np

C = 16
HID = 128
H = W = 256
NB = 33            # row blocks in X layout (33*8 = 264 row slots, rows 0..257 used)
FW = 258           # padded row width in free dim
FSZ = NB * FW      # 8514 free elements per partition
NROW = 256         # image rows (stored at row index 1..256)


def _sobel():
    dx = np.outer([1, 2, 1], [-1, 0, 1]) / 8.0
    f1 = dx.T.astype(np.float32)   # angle=0: F1 = dx.T
    f2 = dx.astype(np.float32)     # F2 = dx
    return f1, f2


def build_weights(W0, b0, W1):
    """Host-side preprocessing of the MLP weights into lhsT tensors."""
    F1, F2 = _sobel()
    W0x, W0y1, W0y2 = W0[:, 0:16], W0[:, 16:32], W0[:, 32:48]
    # A[di][dj]: [HID, C] applied to x[row-1+di, w-1+dj]
    A = [[(np.float32(di == 1 and dj == 1) * W0x
           + F1[di, dj] * W0y1 + F2[di, dj] * W0y2).astype(np.float32)
          for dj in range(3)] for di in range(3)]

    # layer-1 lhsT variants. Key: (s, dj, part) where part is 0 for the lhsT
    # covering the block holding row rho-1 (and possibly rho, rho+1), 1 for
    # the spill into block t+1 (s in {6,7}).
    l1 = {}
    for dj in range(3):
        for s in range(6):
            L = np.zeros((128, 128), np.float32)
            for di in range(3):
                # lhsT[16*(s+di)+c, m] = A[di][dj][m, c]
                L[16 * (s + di):16 * (s + di) + 16, :] = A[di][dj].T
            l1[(s, dj, 0)] = L
        # s in {6, 7}: single matmul against the X2 (4-row-shifted blocking)
        # copy. Row r sits at X2 partition group (r+4)%8: s=6 -> rows at
        # groups 2,3,4; s=7 -> groups 3,4,5.
        for s, g0 in ((6, 2), (7, 3)):
            L = np.zeros((128, 128), np.float32)
            for di in range(3):
                L[16 * (g0 + di):16 * (g0 + di) + 16, :] = A[di][dj].T
            l1[(s, dj, 0)] = L

    # layer-2 lhsT: two M=32 variants shared by all col-groups. A row with
    # group g lands in col-group j=g//2 at partition stripe 32j+16*(g%2).
    # W1E covers even g (stripe low half), W1O odd g (high half); the unused
    # half is zero (accumulates 0 into the partner stripe).
    w1p = []
    for par in range(2):
        Wp = np.zeros((128, 32), np.float32)
        Wp[:, 16 * par:16 * par + 16] = W1.T
        w1p.append(Wp)

    # life-broadcast lhsT variants: for block tb, rows rho=8tb+g (real rows
    # only), life value lives at LifeQ[q = rho-1, half = q//128].
    life_plan = []   # per tb: list of (half, lhsT_index); lhsT K=128 base 0
    r_mats = []
    for tb in range(NB):
        plan = []
        buckets = {}
        for g in range(8):
            rho = 8 * tb + g
            if rho < 1 or rho > 256:
                continue
            q = rho - 1
            half, qh = q // 128, q % 128
            buckets.setdefault(half, []).append((g, qh))
        for half, rows in sorted(buckets.items()):
            # lhsT indexed by the pool tile's q2 layout: row (q%8)*16 + (q//8)%16
            Rm = np.zeros((128, 128), np.float32)
            for g, qh in rows:
                q2 = (qh % 8) * 16 + qh // 8
                Rm[q2, 16 * g:16 * g + 16] = 1.0
            plan.append((half, len(r_mats)))
            r_mats.append(Rm)
        life_plan.append(plan)

    import ml_dtypes
    bf16 = ml_dtypes.bfloat16
    l1_keys = sorted(l1.keys())
    l1_idx = {k: i for i, k in enumerate(l1_keys)}
    l1_stack = np.stack([l1[k] for k in l1_keys]).astype(bf16)  # [NL1,128,128]
    w1_stack = np.stack(w1p).astype(bf16)                  # [2, 128, 32] bf16
    r_stack = np.stack(r_mats).astype(bf16)                # [NR, 128, 128] bf16
    return dict(l1_stack=l1_stack, l1_idx=l1_idx, w1_stack=w1_stack,
                r_stack=r_stack, life_plan=life_plan,
                b0=b0.reshape(128, 1).astype(np.float32))


def marshal_x2(xc):
    """Shifted-blocking copy: row r at partition ((r+4)%8)*16+c, block (r+4)//8."""
    x2 = np.zeros_like(xc)
    x2[64:128, :] = xc[0:64, :]
    x2[0:64, FW:] = xc[64:128, :-FW]
    return x2


def marshal_x(img):
    """[256,256,16] image -> X [128, FSZ] interleaved layout (f32)."""
    xp = np.zeros((NB * 8, FW, C), np.float32)
    xp[1:257, 1:257, :] = img
    # X[g*16+c, t*258+w] = xp[8t+g, w, c]
    xc = xp.reshape(NB, 8, FW, C).transpose(1, 3, 0, 2).reshape(128, FSZ)
    return np.ascontiguousarray(xc)


def unmarshal_x(xc):
    """X [128, FSZ] -> [256,256,16] image."""
    xp = xc.reshape(8, C, NB, FW).transpose(2, 0, 3, 1)   # [NB, 8, FW, C]
    xp = xp.reshape(NB * 8, FW, C)
    return np.ascontiguousarray(xp[1:257, 1:257, :])


def build_program(steps, l1_idx, life_plan, n_l1, n_r,
                  relu_act_ratio=4, defer_tp=3, defer_trios=3):
    """Returns a compiled Bacc ready for run_bass_kernel_spmd.

    relu_act_ratio: of every 4 relu tiles, this many go to ScalarE (rest DVE).
    defer_tp: sweep iteration of step t+1 after which step t's half-1 life
    chain is emitted (so the tensor queue isn't stalled on the post pools).
    """
    import concourse.bass as bass
    import concourse.bacc as bacc
    import concourse.tile as tile
    from concourse import mybir
    F32 = mybir.dt.float32
    BF16 = mybir.dt.bfloat16
    AF = mybir.ActivationFunctionType
    ALU = mybir.AluOpType
    nc = bacc.Bacc(None, target_bir_lowering=False, debug=False, num_devices=8,
                   num_swdge_queues=4)

    xb_ext = nc.declare_dram_parameter("xbc", [128, FSZ], BF16, isOutput=False)
    x2_ext = nc.declare_dram_parameter("x2c", [128, FSZ], BF16, isOutput=False)
    l1_ext = nc.declare_dram_parameter("l1w", [n_l1, 128, 128], BF16, isOutput=False)
    w1_ext = nc.declare_dram_parameter("w1w", [2, 128, 32], BF16, isOutput=False)
    r_ext = nc.declare_dram_parameter("rw", [n_r, 128, 128], BF16, isOutput=False)
    b0_ext = nc.declare_dram_parameter("b0w", [128, 1], F32, isOutput=False)
    out_ext = nc.declare_dram_parameter("out", [128, FSZ], F32, isOutput=True)

    with tile.TileContext(nc) as tc:
        with tc.tile_pool(name="hpool", bufs=12) as hpool, \
             tc.tile_pool(name="ph_pool", bufs=3, space="PSUM") as ph_pool, \
             tc.tile_pool(name="pd_pool", bufs=3, space="PSUM") as pd_pool, \
             tc.tile_pool(name="pl_pool", bufs=2, space="PSUM") as pl_pool:

            # --- persistent state (static SBUF allocations, never released) ---
            XN_A = nc.alloc_sbuf_tensor("XN_A", [128, FSZ], F32).ap()
            XN_B = nc.alloc_sbuf_tensor("XN_B", [128, FSZ], F32).ap()
            Xb_A = nc.alloc_sbuf_tensor("Xb_A", [128, FSZ], BF16).ap()
            Xb_B = nc.alloc_sbuf_tensor("Xb_B", [128, FSZ], BF16).ap()
            X2b = nc.alloc_sbuf_tensor("X2b", [128, FSZ], BF16).ap()

            LW1 = nc.alloc_sbuf_tensor("LW1", [128, n_l1 * 128], BF16).ap()
            LW2 = nc.alloc_sbuf_tensor("LW2", [128, 2 * 32], BF16).ap()
            LWR = nc.alloc_sbuf_tensor("LWR", [128, n_r * 128], BF16).ap()
            l1t = [LW1[:, 128 * i:128 * i + 128] for i in range(n_l1)]
            w1t = [LW2[:, 32 * par:32 * par + 32] for par in range(2)]
            rt = [LWR[:, 128 * i:128 * i + 128] for i in range(n_r)]
            b0t = nc.alloc_sbuf_tensor("b0t", [128, 1], F32).ap()

            A_pre = nc.alloc_sbuf_tensor("A_pre", [128, 516], F32).ap()
            A_post = nc.alloc_sbuf_tensor("A_post", [128, 516], F32).ap()
            HM = nc.alloc_sbuf_tensor("HM", [128, 512], F32).ap()
            HMu = nc.alloc_sbuf_tensor("HMu", [128, 512], F32).ap()
            HMd = nc.alloc_sbuf_tensor("HMd", [128, 512], F32).ap()
            HMp = nc.alloc_sbuf_tensor("HMp", [128, 512], F32).ap()
            HMpu = nc.alloc_sbuf_tensor("HMpu", [128, 512], F32).ap()
            HMpd = nc.alloc_sbuf_tensor("HMpd", [128, 512], F32).ap()
            # seam scratch: partition 0 holds alpha/hm of rows 128,129
            # cols: [0:258] a128 | [258:516] a129 | [516:774] hm128 | [774:1032] hm129
            SEAM = nc.alloc_sbuf_tensor("SEAM", [128, 1032], F32).ap()
            VMpre = nc.alloc_sbuf_tensor("VMpre", [128, 512], F32).ap()
            VMpost = nc.alloc_sbuf_tensor("VMpost", [128, 512], F32).ap()
            LifeQ = nc.alloc_sbuf_tensor("LifeQ", [128, 512], BF16).ap()
            LifeF = nc.alloc_sbuf_tensor("LifeF", [128, 512], F32).ap()
            Zrow = nc.alloc_sbuf_tensor("Zrow", [128, 516], F32).ap()

            # --- PE warm-up ---
            # ~25 dummy matmuls on dependency-free SBUF (Xb_B interior, no
            # writer yet; values irrelevant, result never read) run during
            # the input DMA loads, so the HAM clock-gate reaches 8/8
            # (2.4 GHz) before the first real sweep instead of ramping
            # through ~3.4us of half-rate matmuls.
            pwarm = ph_pool.tile([128, 2, 256], F32, tag="ph")
            for _ in range(25):
                nc.tensor.matmul(
                    pwarm[:], Xb_B[:, 268:396],
                    bass.AP(tensor=Xb_B.tensor, offset=268,
                            ap=[[FSZ, 128], [1, 512]]),
                    start=True, stop=True)

            # --- loads / init ---
            # Ordered so the first sweep iteration unblocks as early as
            # possible: image chunk 0 + layer-1 weights first; the big
            # life-broadcast weights (LWR, only needed mid-sweep) last.
            # Image tensors load in block-chunks so the sweep starts once
            # chunk 0 lands instead of waiting for the full 5MB.
            first = True
            for t0, nt in ((0, 8), (8, 8), (16, 8), (24, 9)):
                lo, n = t0 * FW, nt * FW
                nc.gpsimd.dma_start(out=Xb_A[:, lo:lo + n],
                                    in_=xb_ext[:, lo:lo + n])
                nc.gpsimd.dma_start(out=X2b[:, lo:lo + n],
                                    in_=x2_ext[:, lo:lo + n])
                # f32 master derived on-device from the bf16 input
                nc.vector.tensor_copy(XN_A[:, lo:lo + n], Xb_A[:, lo:lo + n])
                if first:
                    first = False
                    nc.gpsimd.dma_start(out=LW1[:], in_=bass.AP(
                        tensor=l1_ext, offset=0,
                        ap=[[128, 128], [128 * 128, n_l1], [1, 128]]))
                    nc.gpsimd.dma_start(out=b0t[:], in_=b0_ext[:])
                    nc.gpsimd.dma_start(out=LW2[:], in_=bass.AP(
                        tensor=w1_ext, offset=0,
                        ap=[[32, 128], [128 * 32, 2], [1, 32]]))
            nc.gpsimd.dma_start(out=LWR[:], in_=bass.AP(
                tensor=r_ext, offset=0,
                ap=[[128, 128], [128 * 128, n_r], [1, 128]]))
            # only the pad columns of the Xb shadow need zeroing (interior
            # rows are fully rewritten each step; host tensors arrive padded)
            nc.vector.memset(bass.AP(tensor=Xb_B.tensor, offset=0,
                                     ap=[[FSZ, 128], [FW, NB], [257, 2]]), 0.0)
            nc.vector.memset(Zrow[:], 0.0)
            nc.vector.memset(SEAM[0:32, :], 0.0)
            nc.vector.memset(A_post[:], 0.0)
            nc.vector.memset(A_pre[:], 0.0)

            relu_ctr = [0]

            def relu_tile(dst, src):
                use_act = (relu_ctr[0] % 4) < relu_act_ratio
                relu_ctr[0] += 1
                if use_act:
                    nc.scalar.activation(dst, src, AF.Relu, bias=b0t[:], scale=1.0)
                else:
                    nc.vector.tensor_scalar(dst, src, b0t[:], 0.0,
                                            op0=ALU.add, op1=ALU.max)

            def extract_alpha(dst_A, src_X, halves=(0, 1)):
                # q2-layout: dst_A[q2 = gp*16 + j, half*258 + 1 + w] holds
                # alpha of row rho = 128*half + 8j + gp + 1.
                for half in halves:
                    for gp in range(8):
                        g = (gp + 1) % 8
                        t0 = 16 * half + (1 if gp == 7 else 0)
                        dst = bass.AP(
                            tensor=dst_A.tensor,
                            offset=16 * gp * 516 + 258 * half + 1,
                            ap=[[516, 16], [1, 256]])
                        src = bass.AP(
                            tensor=src_X.tensor,
                            offset=(16 * g + 3) * FSZ + t0 * FW + 1,
                            ap=[[FSZ, 1], [FW, 16], [1, 256]])
                        nc.gpsimd.dma_start(out=dst, in_=src)

            def pool_half(dst_VM, src_A, half, hm, hmu, hmd):
                """maxpool one half; seam values come from the SEAM tile."""
                lo, hi = 258 * half, 258 * half + 258
                qlo, qhi = 256 * half, 256 * half + 256
                av = src_A[:, lo:hi]
                nc.vector.tensor_tensor(hm[:, qlo:qhi], av[:, 0:256],
                                        av[:, 2:258], op=ALU.max)
                nc.vector.tensor_tensor(hm[:, qlo:qhi], hm[:, qlo:qhi],
                                        av[:, 1:257], op=ALU.max)
                nc.gpsimd.dma_start(out=hmu[0:112, qlo:qhi], in_=hm[16:128, qlo:qhi])
                nc.gpsimd.dma_start(out=hmu[112:127, qlo:qhi], in_=hm[1:16, qlo:qhi])
                nc.gpsimd.dma_start(out=hmd[16:128, qlo:qhi], in_=hm[0:112, qlo:qhi])
                nc.gpsimd.dma_start(out=hmd[1:16, qlo:qhi], in_=hm[112:127, qlo:qhi])
                if half == 0:
                    # HMu[127, h0] = hm(row 129); HMd[0, h0] = 0
                    nc.gpsimd.dma_start(out=hmu[127:128, 0:256],
                                        in_=SEAM[0:1, 775:1031])
                    nc.gpsimd.dma_start(out=hmd[0:1, 0:256], in_=Zrow[0:1, 0:256])
                else:
                    # HMd[0, h1] = hm(row 128); HMu[127, h1] = 0
                    nc.gpsimd.dma_start(out=hmd[0:1, 256:512],
                                        in_=SEAM[0:1, 517:773])
                    nc.gpsimd.dma_start(out=hmu[127:128, 256:512],
                                        in_=Zrow[0:1, 0:256])
                nc.vector.tensor_tensor(dst_VM[:, qlo:qhi], hm[:, qlo:qhi],
                                        hmu[:, qlo:qhi], op=ALU.max)
                nc.vector.tensor_tensor(dst_VM[:, qlo:qhi], dst_VM[:, qlo:qhi],
                                        hmd[:, qlo:qhi], op=ALU.max)

            def seam_fill_from_A(src_A):
                # rows 128 (q2=127, h0) / 129 (q2=0, h1) from the A tile
                nc.gpsimd.dma_start(out=SEAM[0:1, 1:257],
                                    in_=src_A[127:128, 1:257])
                nc.gpsimd.dma_start(out=SEAM[0:1, 259:515],
                                    in_=src_A[0:1, 259:515])
                sv = SEAM[0:1, :].rearrange("p (a w) -> p a w", a=4)
                nc.vector.tensor_tensor(sv[:, 2:4, 1:257], sv[:, 0:2, 0:256],
                                        sv[:, 0:2, 2:258], op=ALU.max)
                nc.vector.tensor_tensor(sv[:, 2:4, 1:257], sv[:, 2:4, 1:257],
                                        sv[:, 0:2, 1:257], op=ALU.max)

            def seam_fill(src_X):
                # alpha rows 128 (g 0, t 16) and 129 (g 1, t 16) -> SEAM
                # partition 0; then their horizontal maxes.
                nc.gpsimd.dma_start(
                    out=SEAM[0:1, 1:257],
                    in_=bass.AP(tensor=src_X.tensor,
                                offset=3 * FSZ + 16 * FW + 1,
                                ap=[[FSZ, 1], [1, 256]]))
                nc.gpsimd.dma_start(
                    out=SEAM[0:1, 259:515],
                    in_=bass.AP(tensor=src_X.tensor,
                                offset=(16 + 3) * FSZ + 16 * FW + 1,
                                ap=[[FSZ, 1], [1, 256]]))
                sv = SEAM[0:1, :].rearrange("p (a w) -> p a w", a=4)
                nc.vector.tensor_tensor(sv[:, 2:4, 1:257], sv[:, 0:2, 0:256],
                                        sv[:, 0:2, 2:258], op=ALU.max)
                nc.vector.tensor_tensor(sv[:, 2:4, 1:257], sv[:, 2:4, 1:257],
                                        sv[:, 0:2, 1:257], op=ALU.max)

            def xwin2(xt, t, lo, n):
                # two consecutive blocks, same in-block window -> free (2, n)
                return xt.rearrange("p (t f) -> p t f", t=NB)[:, t:t + 2, lo:lo + n]

            def make_life_block(step, XNc, Xbc, last):
                def life_block(tb):
                    lo = tb * FW + 1
                    plan = life_plan[tb]
                    pl = pl_pool.tile([128, 256], F32,
                                      name=f"pl_{step}_{tb}", tag="pl")
                    for i, (half, ridx) in enumerate(plan):
                        nc.tensor.matmul(
                            pl[:], rt[ridx],
                            LifeQ[:, half * 256:half * 256 + 256],
                            start=(i == 0), stop=(i == len(plan) - 1))
                    # life mask applied in place on the f32 master
                    nc.vector.tensor_tensor(XNc[:, lo:lo + 256],
                                            XNc[:, lo:lo + 256], pl[:],
                                            op=ALU.mult)
                    if last:
                        # tail blocks stream out via the ScalarE HWDGE queue
                        # (idle once relu is done); mid-sweep blocks stay on
                        # gpsimd so they don't contend with relu.
                        dq = nc.scalar if tb >= 16 else nc.gpsimd
                        dq.dma_start(
                            out=out_ext[:, lo:lo + 256],
                            in_=XNc[:, lo:lo + 256])
                    else:
                        # bf16 shadow for the next step's matmuls
                        nc.vector.tensor_copy(Xbc[:, lo:lo + 256],
                                              XNc[:, lo:lo + 256])
                        nc.gpsimd.dma_start(
                            out=X2b[64:128, lo:lo + 256],
                            in_=Xbc[0:64, lo:lo + 256])
                        if tb < NB - 1:
                            nc.gpsimd.dma_start(
                                out=X2b[0:64, lo + FW:lo + FW + 256],
                                in_=Xbc[64:128, lo:lo + 256])
                return life_block

            pending_h1 = [None]   # deferred half-1 life chain of prev step

            for step in range(steps):
                last = step + 1 == steps
                if step % 2 == 0:
                    XNprev, XNcur = XN_A, XN_B
                    Xbprev, Xbcur = Xb_A, Xb_B
                else:
                    XNprev, XNcur = XN_B, XN_A
                    Xbprev, Xbcur = Xb_B, Xb_A
                life_block = make_life_block(step, XNcur, Xbcur, last)

                # --- pre pool (A_pre was refreshed at the end of the
                # previous step, off the critical path) ---
                if step == 0:
                    extract_alpha(A_pre, XNprev)
                seam_fill_from_A(A_pre)
                pool_half(VMpre, A_pre, 0, HM, HMu, HMd)
                pool_half(VMpre, A_pre, 1, HM, HMu, HMd)

                # --- main sweep: layer1 + relu + layer2 ---
                d_tiles = {}
                d_count = {}          # per-block total arrivals
                s_count = {}          # per (tb, colgroup j) stripe arrivals
                d_expect = {tb: 8 for tb in range(NB)}
                d_expect[0] = 7
                d_expect[32] = 1
                s_expect = {}
                for tb in range(NB):
                    for j in range(4):
                        s_expect[(tb, j)] = 2
                s_expect[(0, 0)] = 1     # row 0 is padding; only g=1 writes
                s_expect[(32, 0)] = 1    # block 32 holds only row 256 (g=0)

                def post_half(half):
                    extract_alpha(A_post, XNcur, halves=(half,))
                    pool_half(VMpost, A_post, half, HMp, HMpu, HMpd)
                    qlo = 256 * half
                    qs = slice(qlo, qlo + 256)
                    nc.vector.tensor_tensor(LifeF[:, qs], VMpre[:, qs],
                                            VMpost[:, qs], op=ALU.min)
                    nc.vector.tensor_scalar(LifeF[:, qs], LifeF[:, qs],
                                            0.1, None, op0=ALU.is_gt)
                    nc.vector.tensor_copy(LifeQ[:, qs], LifeF[:, qs])
                    if not last:
                        flo = 258 * half + 1
                        nc.vector.tensor_tensor(A_pre[:, flo:flo + 256],
                                                A_post[:, flo:flo + 256],
                                                LifeF[:, qs], op=ALU.mult)
                    if half == 0:
                        for tb in range(0, 16):
                            life_block(tb)

                def l2mm(rho, ht, hslice):
                    # one [K=128, M=32] matmul into col-group stripe 32j of
                    # the block's d tile. The lhsT (W1E/W1O by row parity)
                    # zero-fills the partner 16-row half, so the stripe's
                    # even/odd pair accumulates into disjoint 16-partition
                    # sub-stripes with 2 matmuls per 32-stripe.
                    tb, g = rho // 8, rho % 8
                    j, par = g // 2, g % 2
                    if tb not in d_tiles:
                        d_tiles[tb] = pd_pool.tile([128, 256], F32,
                                                   name=f"pd_s{step}_{tb}",
                                                   tag="pd")
                        d_count[tb] = 0
                    first = s_count.get((tb, j), 0) == 0
                    s_count[(tb, j)] = s_count.get((tb, j), 0) + 1
                    laststripe = s_count[(tb, j)] == s_expect[(tb, j)]
                    nc.tensor.matmul(d_tiles[tb][32 * j:32 * j + 32, :],
                                     w1t[par][:], ht[:, hslice],
                                     start=first, stop=laststripe,
                                     tile_position=(0, 32 * j))
                    d_count[tb] += 1
                    if d_count[tb] == d_expect[tb]:
                        lo = tb * FW + 1
                        nc.vector.tensor_tensor(
                            XNcur[:, lo:lo + 256], d_tiles[tb][:],
                            XNprev[:, lo:lo + 256], op=ALU.add)
                        if tb == 16:
                            seam_fill(XNcur)
                            post_half(0)

                # Layer-2 runs one whole tp behind layer-1 so (a) the tensor
                # queue never waits on relu latency and (b) the 16 matmuls of
                # a tp can be emitted as 4 quads whose members hit 4 distinct
                # 32-column PE groups -> they execute concurrently.
                pending_l2 = []

                def flush_tp_l2():
                    trios = pending_l2[:8]
                    del pending_l2[:8]
                    for seq, hs in (((0, 2, 4, 6), slice(0, 256)),
                                    ((0, 2, 4, 6), slice(256, 512)),
                                    ((1, 3, 5, 7), slice(0, 256)),
                                    ((1, 3, 5, 7), slice(256, 512))):
                        for s in seq:
                            rho, ht = trios[s]
                            l2mm(rho + (8 if hs.start else 0), ht, hs)

                for tp in range(16):
                    if tp == defer_tp and pending_h1[0] is not None:
                        # previous step's half-1 life chain: its LifeQ deps
                        # are long satisfied by now, so these matmuls flow
                        # without stalling the tensor queue.
                        pending_h1[0]()
                        pending_h1[0] = None
                    for s in range(8):
                        rho1 = 16 * tp + s + 1
                        ph = ph_pool.tile([128, 2, 256], F32, tag="ph")
                        src = Xbprev if s <= 5 else X2b
                        tblk = 2 * tp if s <= 5 else 2 * tp + 1
                        for dj in range(3):
                            nc.tensor.matmul(
                                ph[:], l1t[l1_idx[(s, dj, 0)]][:],
                                xwin2(src, tblk, dj, 256),
                                start=(dj == 0), stop=(dj == 2))
                        ht = hpool.tile([128, 512], BF16, tag="ht")
                        relu_tile(ht[:], ph.rearrange("p a b -> p (a b)"))
                        pending_l2.append((rho1, ht))
                        if len(pending_l2) == 8 + defer_trios:
                            flush_tp_l2()
                while pending_l2:
                    flush_tp_l2()

                # h0's post chain was emitted mid-sweep (block 16); finish h1
                post_half(1)

                def deferred(lb=life_block):
                    # block 16 first: the next step's sweep needs its Xb/X2b
                    # updates earliest (tp=7..8 read blocks 16-17).
                    for tb in range(16, NB):
                        lb(tb)
                if last:
                    deferred()
                else:
                    pending_h1[0] = deferred

    nc.compile()
    return nc


_PROGRAM_CACHE = {}


def kernel(x, W0, b0, W1, steps, _trace=False):
    import concourse.bass_utils as bass_utils
    import ml_dtypes
    bf16 = ml_dtypes.bfloat16
    steps = int(steps)
    x = np.asarray(x, dtype=np.float32)
    W0 = np.asarray(W0, dtype=np.float32)
    b0 = np.asarray(b0, dtype=np.float32)
    W1 = np.asarray(W1, dtype=np.float32)
    B = x.shape[0]
    assert x.shape == (8, H, W, C), x.shape

    wts = build_weights(W0, b0, W1)
    key = steps
    if key not in _PROGRAM_CACHE:
        _PROGRAM_CACHE[key] = build_program(steps, wts["l1_idx"],
                                            wts["life_plan"],
                                            wts["l1_stack"].shape[0],
                                            wts["r_stack"].shape[0])
    nc = _PROGRAM_CACHE[key]

    in_maps = []
    for b in range(B):
        xcb = marshal_x(x[b])
        xbb = xcb.astype(bf16)
        in_maps.append({
            "xbc": xbb,
            "x2c": marshal_x2(xbb),
            "l1w": wts["l1_stack"],
            "w1w": wts["w1_stack"],
            "rw": wts["r_stack"],
            "b0w": wts["b0"],
        })
    res = bass_utils.run_bass_kernel_spmd(nc, in_maps, list(range(8)),
                                          trace=_trace)
    kernel.last_result = res
    out = np.stack([unmarshal_x(res.results[b]["out"]) for b in range(B)])
    return out.astype(np.float32)

